# revision 1
# baseline (speedup 1.0000x reference)
"""nn_MultiHeadAttention_59253368815813 on 8 TRN2 NeuronCores.

The reference module is bug-faithful to its original nn.Module in two ways
that together collapse the computation:

  1. ``o = jnp.einsum('bhtl,bthd->bhtd', A, v)`` indexes ``v`` by the QUERY
     position ``t``, not the key position ``l``. ``l`` therefore only sums
     over the softmax weights, which sum to exactly 1 per row:
     ``o[b,h,t,d] == v[b,t,h,d]``. Q, K, the mask and the softmax never
     influence the output (verified vs the reference to 4e-7 rel).
  2. ``o.reshape(b, T, d)`` with no transpose scrambles (head, token) so the
     reshaped activation row tj = 128*h + s is the concatenation over
     m=0..15 of v[b, 16*s+m, h, :].

So the exact computation is  out = scramble(x @ Wv) @ Wo.T,  and the
scramble makes output rows depend on one head only. Sharding: core c owns
heads {2c, 2c+1}, i.e. Wv columns [128c, 128c+128) and output rows
[256c, 256c+256) of each batch; the host concatenates the row slabs.
No cross-core reduction needed.

Per core (fp32r matmuls, fp32 PSUM):
  vT[128ch, u] = Wv_slice^T @ x^T   where the host feeds x^T with tokens
  permuted to u = m*128 + r (t = 16r + m), so the reshape scramble becomes
  contiguous: the PSUM evacuation writes vt2[64*(m%2)+di, (m//2)*128+r] and
  the output projection is 8 accumulating K=128 matmuls per output tile:
  out[128h + r, n] = sum_m2 vt2_chunk(m2)^T @ WoT[128*m2:128*m2+128, n].
"""

import sys
import types

import numpy as np

_TRN_REPO = "/opt/trn_rl_repo"
if _TRN_REPO not in sys.path:
    sys.path.insert(0, _TRN_REPO)


def _install_ntff_shim():
    """antenv.axon_hooks is absent in this container; provide it so
    BASS_TRACE=1 profiling works. No-op if the real module exists."""
    try:
        import antenv  # noqa: F401
    except ImportError:
        return
    if "antenv.axon_hooks" in sys.modules:
        return
    try:
        import antenv.axon_hooks  # noqa: F401
        return
    except ImportError:
        pass
    m = types.ModuleType("antenv.axon_hooks")
    m._hook = None
    m.set_axon_ntff_profile_hook = lambda h: setattr(m, "_hook", h)
    m.get_axon_ntff_profile_hook = lambda: m._hook
    sys.modules["antenv.axon_hooks"] = m
    try:
        from trn_agent_boot.trn_boot import _ntff_profile_via_ctypes

        hook = _ntff_profile_via_ctypes("/opt/axon/libaxon_pjrt.so")
        if hook is not None:
            m.set_axon_ntff_profile_hook(hook)
    except Exception:
        pass


_install_ntff_shim()

import concourse.mybir as mybir  # noqa: E402
import concourse.tile as tile  # noqa: E402
from concourse import bacc  # noqa: E402
from concourse.bass_utils import run_bass_kernel_spmd  # noqa: E402

F32 = mybir.dt.float32
F32R = mybir.dt.float32r

B = 2
T = 2048
D = 1024
NCORES = 8
GT = B * T          # 4096
NG = GT // 512      # 8 global 512-token chunks
NDCH = D // 128     # 8 contraction chunks for the projection

_CACHED = None
LAST_RESULTS = None


def _build_module():
    nc = bacc.Bacc("TRN2", target_bir_lowering=False, debug=False,
                   num_devices=NCORES)

    xT_d = nc.dram_tensor("xT", [D, GT], F32R, kind="ExternalInput").ap()
    wv_d = nc.dram_tensor("wv", [128, NDCH, 128], F32R,
                          kind="ExternalInput").ap()
    wo_d = nc.dram_tensor("woT", [128, 8, D], F32R,
                          kind="ExternalInput").ap()
    out_d = nc.dram_tensor("out", [B, 256, D], F32, kind="ExternalOutput").ap()

    with tile.TileContext(nc) as tc:
        _emit(nc, tc, xT_d, wv_d, wo_d, out_d)
    nc.compile()
    return nc


def _emit(nc, tc, xT_d, wv_d, wo_d, out_d):
    from contextlib import ExitStack

    ctx = ExitStack()
    with ctx:
        wpool = ctx.enter_context(tc.tile_pool(name="w", bufs=1))
        xtp = ctx.enter_context(tc.tile_pool(name="xt", bufs=4))
        vtp = ctx.enter_context(tc.tile_pool(name="vt", bufs=1))
        outp = ctx.enter_context(tc.tile_pool(name="outsb", bufs=4))
        ps_p = ctx.enter_context(tc.tile_pool(name="ps_p", bufs=4, space="PSUM"))
        ps_w = ctx.enter_context(tc.tile_pool(name="ps_w", bufs=3, space="PSUM"))

        # weights ride the ACT HWDGE ring; activations the SP ring (parallel)
        wv_sb = wpool.tile([128, NDCH, 128], F32R, tag="wv")
        nc.scalar.dma_start(wv_sb[:], wv_d)
        wo_sb = wpool.tile([128, 8, D], F32R, tag="wo")

        # vt2[h][64*(m%2)+di, b*1024 + (m//2)*128 + r] = v[b, t=16r+m, 64h+di]
        vt = [vtp.tile([128, GT // 2], F32R, tag=f"vt{h}", name=f"vt{h}")
              for h in range(2)]

        def proj_half(half, before_j=None):
            """v^T for one 2048-token half (= one batch). before_j(j) lets the
            caller interleave other PE work between the 2MB-chunk groups."""
            pss = [ps_p.tile([128, 512], F32, tag="proj",
                             name=f"psp{half}_{q}") for q in range(4)]
            for j in range(4):
                xt = xtp.tile([128, 2, 2048], F32R, tag="xt",
                              name=f"xt{half}_{j}")
                nc.sync.dma_start(
                    xt[:], xT_d[j * 256:(j + 1) * 256,
                                half * 2048:(half + 1) * 2048]
                    .rearrange("(ko ki) t -> ki ko t", ki=128))
                if before_j is not None:
                    before_j(j)
                for kk in range(2):
                    dch = 2 * j + kk
                    for q in range(4):
                        nc.tensor.matmul(pss[q][:], wv_sb[:, dch, :],
                                         xt[:, kk, q * 512:(q + 1) * 512],
                                         start=(dch == 0),
                                         stop=(dch == NDCH - 1))

            for q in range(4):
                for h in range(2):
                    for mm in range(4):
                        m = q * 4 + mm
                        j, m2 = m % 2, m // 2
                        nc.vector.tensor_copy(
                            vt[h][64 * j:64 * j + 64,
                                  half * 1024 + m2 * 128:
                                  half * 1024 + (m2 + 1) * 128],
                            pss[q][64 * h:64 * h + 64,
                                   mm * 128:(mm + 1) * 128])

        def wo_block(b, h, nch):
            """Output rows [128h, 128h+128) of batch b, cols [512nch, +512)."""
            ps = ps_w.tile([128, 512], F32, tag="wo", name=f"psw{b}_{h}_{nch}")
            for m2 in range(8):
                lhs = vt[h][:, b * 1024 + m2 * 128:b * 1024 + (m2 + 1) * 128]
                nc.tensor.matmul(ps[:], lhs,
                                 wo_sb[:, m2, nch * 512:(nch + 1) * 512],
                                 start=(m2 == 0), stop=(m2 == 7))
            ob = outp.tile([128, 512], F32, tag="ob", name=f"ob{b}_{h}_{nch}")
            nc.vector.tensor_copy(ob[:], ps[:])
            nc.scalar.dma_start(
                out_d[b, 128 * h:128 * h + 128,
                      nch * 512:(nch + 1) * 512], ob[:])

        proj_half(0)
        # woT rides the sync ring between the two xt halves: it doesn't steal
        # bandwidth from xt(half0, 0) and still lands before the first wo_block
        nc.sync.dma_start(wo_sb[:], wo_d)
        # during half-1's DMA stream, fill PE gaps with batch-0 out-proj
        proj_half(1, before_j=lambda j: wo_block(0, j // 2, j % 2))
        for h in range(2):
            for nch in range(2):
                wo_block(1, h, nch)


def _get_module():
    global _CACHED
    if _CACHED is None:
        _CACHED = _build_module()
    return _CACHED


def _round_f32r(a):
    """Round fp32 to the fp32r grid (RNE at 11 mantissa bits) — verified
    bit-identical to the hardware fp32->fp32r cast."""
    b = np.ascontiguousarray(a, np.float32).view(np.uint32).astype(np.uint64)
    lsb = (b >> 12) & 1
    out = (b + 0x7FF + lsb) & np.uint64(0xFFFFF000)
    return out.astype(np.uint32).view(np.float32)


def kernel(x, mask, Wq, Wk, Wv, Wo):
    global LAST_RESULTS
    x = np.asarray(x, dtype=np.float32)
    Wv = np.asarray(Wv, dtype=np.float32)
    Wo = np.asarray(Wo, dtype=np.float32)

    b, t, d = x.shape
    assert (b, t, d) == (B, T, D), (b, t, d)

    # x^T with tokens permuted to u = m*128 + r  (original t = 16r + m)
    xT = x.transpose(2, 0, 1).reshape(D, B, 128, 16)
    xT = _round_f32r(xT.swapaxes(2, 3).reshape(D, GT))
    # woT[p, m2, n] = Wo.T[128*m2 + p, n]
    woT = _round_f32r(Wo.T.reshape(8, 128, D).transpose(1, 0, 2))
    wv_r = _round_f32r(Wv)

    in_maps = []
    for c in range(NCORES):
        wv_c = wv_r[:, 128 * c:128 * c + 128]  # [1024, 128]
        wv_c = np.ascontiguousarray(
            wv_c.reshape(NDCH, 128, 128).transpose(1, 0, 2))
        in_maps.append({
            "xT": xT,
            "woT": woT,
            "wv": wv_c,
        })

    nc = _get_module()
    res = run_bass_kernel_spmd(nc, in_maps, list(range(NCORES)))
    LAST_RESULTS = res
    out = np.concatenate([res.results[c]["out"] for c in range(NCORES)],
                         axis=1)
    return np.ascontiguousarray(out.astype(np.float32))



# revision 7
# speedup vs baseline: 1.5531x; 1.5531x over previous
"""nn_MultiHeadAttention_59253368815813 on 8 TRN2 NeuronCores.

The reference module is bug-faithful to its original nn.Module in two ways
that together collapse the computation:

  1. ``o = jnp.einsum('bhtl,bthd->bhtd', A, v)`` indexes ``v`` by the QUERY
     position ``t``, not the key position ``l``. ``l`` therefore only sums
     over the softmax weights, which sum to exactly 1 per row:
     ``o[b,h,t,d] == v[b,t,h,d]``. Q, K, the mask and the softmax never
     influence the output.
  2. ``o.reshape(b, T, d)`` with no transpose scrambles (head, token): the
     reshaped activation row r = 128*h + s (s = t//16) takes column block
     m = t%16, i.e. Vscr[b][128h+s, 64m+dk] = (x@Wv)[b, 16s+m, 64h+dk].

So the exact computation is  out = Vscr @ Wo.T  with Vscr the scrambled
x@Wv.  Sharding (4 head-groups x 2 s-groups = 8 cores): core (j, g) owns
heads {4j..4j+3} (Wv cols [256j, 256j+256)) and s in [64g, 64g+64)
(tokens [1024g, 1024g+1024) per batch). Each core produces out rows
{128h + s : owned h, s} for both batches; the host reassembles. No
cross-core reduction.

All operands travel as bf16 (rel-err budget 2e-2; bf16 end-to-end lands
~3e-3) which both halves DMA and doubles PE rate vs fp32r (fp32r measures
~466ns per 512-row matmul on HW; bf16 ~216ns).

Per-core on-chip schedule, with token columns host-permuted to
u = m*128 + b*64 + s_lo so every shuffle copy is a contiguous block:
  vT[c, u] = Wv_slice^T @ xT           (4 N-groups of 512 tokens, 8 K)
  shuffle: VscrT[64(m%2)+dk, (b,q,hl,s)] = vT[64hl+dk, (m,b,s)]
           64 copies of [64p x 128] spread over DVE/Act/Pool, pipelined
           behind the v-proj N-groups
  out[64hl+s, n] = sum_k VscrT_k^T @ WoT_k   (8 accumulating matmuls per
           [128, 512] psum tile; 8 tiles in 2 waves of 4 banks)
PE is kept dense: warm-up matmuls on zeroed SBUF bridge the DMA startup
so the tensor engine reaches its top p-state before real work arrives.
"""

import sys
import types

import numpy as np

_TRN_REPO = "/opt/trn_rl_repo"
if _TRN_REPO not in sys.path:
    sys.path.insert(0, _TRN_REPO)


def _install_ntff_shim():
    """antenv.axon_hooks is absent in this container; provide it so
    BASS_TRACE=1 profiling works. No-op if the real module exists."""
    try:
        import antenv  # noqa: F401
    except ImportError:
        return
    if "antenv.axon_hooks" in sys.modules:
        return
    try:
        import antenv.axon_hooks  # noqa: F401
        return
    except ImportError:
        pass
    m = types.ModuleType("antenv.axon_hooks")
    m._hook = None
    m.set_axon_ntff_profile_hook = lambda h: setattr(m, "_hook", h)
    m.get_axon_ntff_profile_hook = lambda: m._hook
    sys.modules["antenv.axon_hooks"] = m
    try:
        from trn_agent_boot.trn_boot import _ntff_profile_via_ctypes

        hook = _ntff_profile_via_ctypes("/opt/axon/libaxon_pjrt.so")
        if hook is not None:
            m.set_axon_ntff_profile_hook(hook)
    except Exception:
        pass


_install_ntff_shim()

import ml_dtypes  # noqa: E402

import concourse.mybir as mybir  # noqa: E402
import concourse.tile as tile  # noqa: E402
from concourse import bacc  # noqa: E402
from concourse.bass_utils import run_bass_kernel_spmd  # noqa: E402

F32 = mybir.dt.float32
BF16 = mybir.dt.bfloat16
BF16_NP = np.dtype(ml_dtypes.bfloat16)

B = 2
T = 2048
D = 1024
NCORES = 8
HG = 4              # head groups (4 heads each)
SG = 2              # s groups (64 s-values each)
TOK = B * T // SG   # token columns per core = 2048
NWARM = 16          # PE warm-up matmuls to bridge DMA startup

_CACHED = None
LAST_RESULTS = None


def _build_module():
    nc = bacc.Bacc("TRN2", target_bir_lowering=False, debug=False,
                   num_devices=NCORES)

    xT_d = nc.dram_tensor("xT", [D, TOK], BF16, kind="ExternalInput").ap()
    wv_d = nc.dram_tensor("wv", [128, 8, 2, 128], BF16,
                          kind="ExternalInput").ap()
    wo_d = nc.dram_tensor("woT", [128, 8, D], BF16,
                          kind="ExternalInput").ap()
    out_d = nc.dram_tensor("out", [B, 2, 128, D], BF16,
                           kind="ExternalOutput").ap()

    with tile.TileContext(nc) as tc:
        _emit(nc, tc, xT_d, wv_d, wo_d, out_d)
    nc.compile()
    return nc


def _emit(nc, tc, xT_d, wv_d, wo_d, out_d):
    from contextlib import ExitStack

    ctx = ExitStack()
    with ctx:
        wpool = ctx.enter_context(tc.tile_pool(name="w", bufs=1))
        xtp = ctx.enter_context(tc.tile_pool(name="xt", bufs=4))
        vsp = ctx.enter_context(tc.tile_pool(name="vscr", bufs=8))
        outp = ctx.enter_context(tc.tile_pool(name="outsb", bufs=4))
        wmp = ctx.enter_context(tc.tile_pool(name="warm", bufs=2))
        ps_v = ctx.enter_context(tc.tile_pool(name="ps_v", bufs=4,
                                              space="PSUM"))
        ps_o = ctx.enter_context(tc.tile_pool(name="ps_o", bufs=4,
                                              space="PSUM"))

        # --- PE warm-up: keep the tensor engine busy (and ramping to its
        # top p-state) while the first DMAs are in flight. Warm matmuls
        # target the first real v-proj psum tile; its start=True reset
        # discards them, and the tile has real readers (the shuffle).
        wa = wmp.tile([128, 128], BF16, tag="wa")
        wb = wmp.tile([128, 512], BF16, tag="wb")
        nc.gpsimd.memset(wa[:], 0)
        nc.gpsimd.memset(wb[:], 0)
        psv0 = [ps_v.tile([128, 512], F32, tag="proj",
                          name=f"psv0_{ct}") for ct in range(2)]
        for i in range(NWARM):
            nc.tensor.matmul(psv0[0][:], wa[:], wb[:], start=True, stop=True,
                             skip_group_check=True)

        # --- input DMAs: weights on the ACT ring, activations on SP ring
        wv_sb = wpool.tile([128, 8, 2, 128], BF16, tag="wv")
        nc.scalar.dma_start(wv_sb[:], wv_d)
        xt = []
        for a in range(4):
            t = xtp.tile([128, 8, 512], BF16, tag="xt", name=f"xt{a}")
            nc.sync.dma_start(
                t[:], xT_d[:, a * 512:(a + 1) * 512]
                .rearrange("(ko ki) u -> ki ko u", ki=128))
            xt.append(t)
        wo_sb = wpool.tile([128, 8, D], BF16, tag="wo")
        nc.scalar.dma_start(wo_sb[:], wo_d)

        # VscrT tiles: [p=(m%2,dk), b, q, hl, s]
        vscr = [vsp.tile([128, 2, 2, 2, 64], BF16, tag=f"v{k}",
                         name=f"vscr{k}") for k in range(8)]

        # GPSIMD cannot read PSUM; only DVE and Act can evacuate it
        copy_engines = [nc.vector.tensor_copy,
                        nc.scalar.copy]
        cp_i = 0

        def shuffle(a, psv):
            """Evacuate v-proj psum tiles of N-group a into VscrT (bf16)."""
            nonlocal cp_i
            for mrel in range(4):
                m = 4 * a + mrel
                k, m2 = m // 2, m % 2
                for ct in range(2):
                    for hl in range(2):
                        src = psv[ct][64 * hl:64 * hl + 64,
                                      mrel * 128:(mrel + 1) * 128] \
                            .rearrange("p (b s) -> p b s", b=2)
                        dst = vscr[k][64 * m2:64 * m2 + 64, :, ct, hl, :]
                        copy_engines[cp_i % 2](dst, src)
                        cp_i += 1

        def vproj(a, psv=None):
            if psv is None:
                psv = [ps_v.tile([128, 512], F32, tag="proj",
                                 name=f"psv{a}_{ct}") for ct in range(2)]
            for ct in range(2):
                for k in range(8):
                    nc.tensor.matmul(psv[ct][:], wv_sb[:, k, ct, :],
                                     xt[a][:, k, :],
                                     start=(k == 0), stop=(k == 7),
                                     skip_group_check=(a == 0))
            return psv

        # wave-1 out-proj (batch 0): 4 psum tiles accumulated k-major so the
        # PE can chase the shuffle pipeline
        po1 = None

        def w1_tiles():
            nonlocal po1
            po1 = [ps_o.tile([128, 512], F32, tag="wo",
                             name=f"po0_{q}_{n}")
                   for q in range(2) for n in range(2)]

        def w1_k(k):
            for q in range(2):
                for n in range(2):
                    nc.tensor.matmul(
                        po1[2 * q + n][:], vscr[k][:, 0, q, :, :],
                        wo_sb[:, k, n * 512:(n + 1) * 512],
                        start=(k == 0), stop=(k == 7),
                        skip_group_check=True)

        def evac(ps, b, q, n, eng):
            ob = outp.tile([128, 512], BF16, tag="ob", name=f"ob{b}_{q}_{n}")
            eng(ob[:], ps[:])
            nc.sync.dma_start(out_d[b, q, :, n * 512:(n + 1) * 512], ob[:])

        # ---- pipeline ----
        psv0 = vproj(0, psv0)
        psv1 = vproj(1)
        shuffle(0, psv0)
        w1_tiles()
        w1_k(0)
        psv2 = vproj(2)
        shuffle(1, psv1)
        w1_k(1)
        w1_k(2)
        psv3 = vproj(3)
        shuffle(2, psv2)
        w1_k(3)
        w1_k(4)
        w1_k(5)
        shuffle(3, psv3)
        w1_k(6)
        w1_k(7)
        for q in range(2):
            for n in range(2):
                evac(po1[2 * q + n], 0, q, n,
                     nc.vector.tensor_copy if n == 0 else nc.scalar.copy)
        # wave-2 (batch 1): tile-major so evacuation pipelines
        for q in range(2):
            for n in range(2):
                po = ps_o.tile([128, 512], F32, tag="wo", name=f"po1_{q}_{n}")
                for k in range(8):
                    nc.tensor.matmul(po[:], vscr[k][:, 1, q, :, :],
                                     wo_sb[:, k, n * 512:(n + 1) * 512],
                                     start=(k == 0), stop=(k == 7))
                evac(po, 1, q, n,
                     nc.vector.tensor_copy if n == 0 else nc.scalar.copy)


def _get_module():
    global _CACHED
    if _CACHED is None:
        _CACHED = _build_module()
    return _CACHED


def kernel(x, mask, Wq, Wk, Wv, Wo):
    global LAST_RESULTS
    x = np.asarray(x, dtype=np.float32)
    Wv = np.asarray(Wv, dtype=np.float32)
    Wo = np.asarray(Wo, dtype=np.float32)

    b, t, d = x.shape
    assert (b, t, d) == (B, T, D), (b, t, d)

    # x^T slabs per s-group, token columns permuted to u = m*128 + b*64 + s_lo
    # (original t = 16*(64g + s_lo) + m)
    xTs = []
    for g in range(SG):
        xs = x[:, 1024 * g:1024 * (g + 1), :]          # [b, 1024, d]
        xs = xs.reshape(B, 64, 16, D).transpose(3, 2, 0, 1)  # [d, m, b, s]
        xTs.append(np.ascontiguousarray(xs.reshape(D, TOK)).astype(BF16_NP))

    # woT[p=(m%2,dk), k, n] = Wo.T[64*(2k+m%2)+dk, n]
    woT = Wo.T.reshape(8, 2, 64, D).transpose(1, 2, 0, 3)
    woT = np.ascontiguousarray(woT.reshape(128, 8, D)).astype(BF16_NP)

    in_maps = []
    for c in range(NCORES):
        j, g = c // SG, c % SG
        wv_c = Wv[:, 256 * j:256 * j + 256]
        wv_c = np.ascontiguousarray(
            wv_c.reshape(8, 128, 2, 128).transpose(1, 0, 2, 3)
        ).astype(BF16_NP)
        in_maps.append({"xT": xTs[g], "woT": woT, "wv": wv_c})

    nc = _get_module()
    res = run_bass_kernel_spmd(nc, in_maps, list(range(NCORES)))
    LAST_RESULTS = res

    out = np.empty((B, T, D), dtype=np.float32)
    F = out.reshape(B, 16, 2, 64, D)        # (b, h, g, s_lo, n)
    for c in range(NCORES):
        j, g = c // SG, c % SG
        ob = np.asarray(res.results[c]["out"]).astype(np.float32)
        F[:, 4 * j:4 * j + 4, g] = ob.reshape(B, 4, 64, D)
    return out
